# revision 16
# baseline (speedup 1.0000x reference)
"""Multi-head self-attention (B=4, T=2048, D=1024, H=16) on 8 TRN2 NeuronCores.

Sharding: tensor-parallel over heads. Core c owns heads (2c, 2c+1):
  - W_Q/W_K/W_V rows [128c, 128c+128) -> per-core q/k/v of shape [T*B, 128]
  - attention for its 2 heads (causal, block-skipped)
  - partial output projection through W_O columns [128c, 128c+128)
Host sums the 8 partial outputs (the row-parallel W_O reduction).

Layouts (on device, per core):
  xT   [8, 128, 8192]  : x^T tiled over model dim (bf16)
  qT/kT [128, 8192]    : per-head-pair transposed q/k (partition = head dim)
  vaug [128, 64, 130]  : v in token-major 128-tiles, per head [64 dims | ones]
  scores^T tiles [128 k-tok, 512 q-tok] so the AV matmul contracts k on
  partitions; softmax denominator = ones-column row of the AV output.
"""

import os
import sys

import numpy as np

if "/opt/trn_rl_repo" not in sys.path:
    sys.path.insert(0, "/opt/trn_rl_repo")

import ml_dtypes

B, T, D, NH, DH = 4, 2048, 1024, 16, 64
NT = B * T          # 8192 tokens
MT = D // 128       # 8 model-dim tiles
NCH = NT // 512     # 16 token chunks
N_CORES = 8

_cache = {}


def _build_nc():
    from contextlib import ExitStack

    import concourse.mybir as mybir
    import concourse.tile as tile
    from concourse import bacc

    BF = mybir.dt.bfloat16
    F32 = mybir.dt.float32
    EXP = mybir.ActivationFunctionType.Exp

    nc = bacc.Bacc("TRN2", target_bir_lowering=False, debug=False)

    xT_d = nc.dram_tensor("xT", [MT, 128, NT], BF, kind="ExternalInput")
    wq_d = nc.dram_tensor("wqT", [MT, 128, 128], BF, kind="ExternalInput")
    wk_d = nc.dram_tensor("wkT", [MT, 128, 128], BF, kind="ExternalInput")
    wv_d = nc.dram_tensor("wvT", [MT, 128, 128], BF, kind="ExternalInput")
    wo_d = nc.dram_tensor("woT", [128, D], BF, kind="ExternalInput")
    cm_d = nc.dram_tensor("cmask", [4, 128, 512], F32, kind="ExternalInput")
    out_d = nc.dram_tensor("out", [NT, D], F32, kind="ExternalOutput")

    with tile.TileContext(nc) as tc, ExitStack() as ctx:
        pers = ctx.enter_context(tc.tile_pool(name="pers", bufs=1))
        qT = pers.tile([128, NT], BF)
        kT = pers.tile([128, NT], BF)
        vaug = pers.tile([128, 64, 130], BF)
        masks = pers.tile([128, 4, 512], F32)
        wq = pers.tile([128, MT, 128], BF)
        wk = pers.tile([128, MT, 128], BF)
        wv = pers.tile([128, MT, 128], BF)
        wo = pers.tile([128, D], BF)

        ones64 = pers.tile([1, 64], BF)
        nc.vector.memset(ones64, 1.0)
        nc.vector.memset(vaug, 1.0)
        for i in range(4):
            nc.sync.dma_start(out=masks[:, i, :], in_=cm_d[i])
        for mt in range(MT):
            nc.sync.dma_start(out=wq[:, mt, :], in_=wq_d[mt])
            nc.sync.dma_start(out=wk[:, mt, :], in_=wk_d[mt])
            nc.sync.dma_start(out=wv[:, mt, :], in_=wv_d[mt])
        nc.sync.dma_start(out=wo[:], in_=wo_d[:])

        # ---- Phase 1: q/k/v projections, one 512-token chunk at a time ----
        with tc.tile_pool(name="xc", bufs=3) as xpool, \
             tc.tile_pool(name="pq", bufs=2, space="PSUM") as pq_pool, \
             tc.tile_pool(name="pk", bufs=2, space="PSUM") as pk_pool, \
             tc.tile_pool(name="pv", bufs=2, space="PSUM") as pv_pool:
            for c in range(NCH):
                cs = slice(c * 512, (c + 1) * 512)
                xc = xpool.tile([128, MT, 512], BF)
                for mt in range(MT):
                    nc.sync.dma_start(out=xc[:, mt, :], in_=xT_d[mt, :, cs])
                pq = pq_pool.tile([128, 512], F32)
                pk = pk_pool.tile([128, 512], F32)
                pv = pv_pool.tile([128, 4, 128], F32)
                for mt in range(MT):
                    nc.tensor.matmul(pq, wq[:, mt, :], xc[:, mt, :],
                                     start=(mt == 0), stop=(mt == MT - 1))
                for mt in range(MT):
                    nc.tensor.matmul(pk, wk[:, mt, :], xc[:, mt, :],
                                     start=(mt == 0), stop=(mt == MT - 1))
                for tt in range(4):
                    for mt in range(MT):
                        nc.tensor.matmul(pv[:, tt, :],
                                         xc[:, mt, tt * 128:(tt + 1) * 128],
                                         wv[:, mt, :],
                                         start=(mt == 0), stop=(mt == MT - 1))
                nc.vector.tensor_copy(out=qT[:, cs], in_=pq)
                nc.vector.tensor_copy(out=kT[:, cs], in_=pk)
                for tt in range(4):
                    t = c * 4 + tt
                    nc.vector.tensor_copy(out=vaug[:, t, 0:64], in_=pv[:, tt, 0:64])
                    nc.vector.tensor_copy(out=vaug[:, t, 65:129], in_=pv[:, tt, 64:128])

        # ---- Phase 2: causal attention + partial output projection ----
        with tc.tile_pool(name="ps_s", bufs=3, space="PSUM") as sp, \
             tc.tile_pool(name="ps_av", bufs=2, space="PSUM") as avp, \
             tc.tile_pool(name="ps_bc", bufs=1, space="PSUM") as bcp, \
             tc.tile_pool(name="ps_o", bufs=2, space="PSUM") as op_, \
             tc.tile_pool(name="esb", bufs=4) as ep, \
             tc.tile_pool(name="nrm", bufs=3) as nrm, \
             tc.tile_pool(name="osb", bufs=3) as osb, \
             tc.tile_pool(name="hop", bufs=2) as hop:
            for b in range(B):
                hoT = hop.tile([128, T], BF)
                for h in range(2):
                    hp = 64 * h
                    for qc in range(4):
                        qoff = b * T + qc * 512
                        pav = avp.tile([65, 512], F32)
                        nkt = 4 * (qc + 1)
                        for kt in range(nkt):
                            koff = b * T + kt * 128
                            pss = sp.tile([128, 512], F32)
                            nc.tensor.matmul(
                                pss,
                                kT[hp:hp + 64, koff:koff + 128],
                                qT[hp:hp + 64, qoff:qoff + 512],
                                start=True, stop=True)
                            if kt >= 4 * qc:
                                nc.vector.tensor_add(pss, pss,
                                                     masks[:, kt - 4 * qc, :])
                            ex = ep.tile([128, 512], BF)
                            nc.scalar.activation(out=ex, in_=pss, func=EXP,
                                                 scale=0.125)
                            nc.tensor.matmul(
                                pav,
                                vaug[:, b * 16 + kt, 65 * h:65 * h + 65],
                                ex,
                                start=(kt == 0), stop=(kt == nkt - 1))
                        inv = nrm.tile([1, 512], BF)
                        with nc.allow_low_precision(
                                reason="softmax denom reciprocal to bf16"):
                            nc.vector.reciprocal(out=inv, in_=pav[64:65, :])
                        pbc = bcp.tile([64, 512], F32)
                        nc.tensor.matmul(pbc, ones64[:], inv[:],
                                         start=True, stop=True)
                        invb = nrm.tile([64, 512], F32)
                        nc.vector.tensor_copy(out=invb, in_=pbc)
                        nc.vector.tensor_mul(
                            hoT[hp:hp + 64, qc * 512:(qc + 1) * 512],
                            pav[0:64, :], invb)
                for tt in range(T // 128):
                    to = b * T + tt * 128
                    for oc in range(2):
                        po = op_.tile([128, 512], F32)
                        nc.tensor.matmul(po, hoT[:, tt * 128:(tt + 1) * 128],
                                         wo[:, oc * 512:(oc + 1) * 512],
                                         start=True, stop=True)
                        ost = osb.tile([128, 512], F32)
                        nc.vector.tensor_copy(out=ost, in_=po)
                        nc.sync.dma_start(
                            out=out_d[to:to + 128, oc * 512:(oc + 1) * 512],
                            in_=ost)
    nc.compile()
    return nc


def _get_nc():
    if "nc" not in _cache:
        _cache["nc"] = _build_nc()
    return _cache["nc"]


def _bf(a):
    return np.ascontiguousarray(a, dtype=np.float32).astype(ml_dtypes.bfloat16)


def make_in_maps(x, W_Q, W_K, W_V, W_O):
    xT = _bf(x.reshape(NT, D).T).reshape(MT, 128, NT)
    cmask = np.full((4, 128, 512), -1e10, dtype=np.float32)
    for t in range(4):
        for kp in range(128):
            cmask[t, kp, t * 128 + kp:] = 0.0
    in_maps = []
    for c in range(N_CORES):
        rs = slice(c * 128, (c + 1) * 128)
        in_maps.append({
            "xT": xT,
            "wqT": _bf(W_Q[rs, :].T).reshape(MT, 128, 128),
            "wkT": _bf(W_K[rs, :].T).reshape(MT, 128, 128),
            "wvT": _bf(W_V[rs, :].T).reshape(MT, 128, 128),
            "woT": _bf(W_O[:, rs].T),
            "cmask": cmask,
        })
    return in_maps


def _ensure_ntff_hook():
    """Install antenv.axon_hooks shim (missing in this image) so
    run_bass_kernel_spmd(trace=True) can capture NTFF profiles."""
    try:
        from antenv import axon_hooks  # noqa: F401
        return True
    except ImportError:
        pass
    try:
        import contextlib
        import ctypes
        import types

        import antenv

        so_path = "/opt/axon/libaxon_pjrt.so"
        lib = ctypes.CDLL(so_path)
        if not hasattr(lib, "axon_start_nrt_profile"):
            return False
        lib.axon_start_nrt_profile.argtypes = [
            ctypes.POINTER(ctypes.c_int64), ctypes.c_size_t]
        lib.axon_start_nrt_profile.restype = ctypes.c_int64
        lib.axon_stop_nrt_profile.argtypes = [ctypes.c_char_p]
        lib.axon_stop_nrt_profile.restype = ctypes.c_int64

        @contextlib.contextmanager
        def _hook(output_dir, device_ids):
            import jax
            jax.devices()
            if device_ids:
                ids = (ctypes.c_int64 * len(device_ids))(*device_ids)
                rc = lib.axon_start_nrt_profile(ids, len(device_ids))
            else:
                rc = lib.axon_start_nrt_profile(None, 0)
            if rc != 0:
                raise RuntimeError(f"axon_start_nrt_profile rc={rc}")
            try:
                yield
            finally:
                n = lib.axon_stop_nrt_profile(str(output_dir).encode())
                print(f"ntff profile: {n} file(s) -> {output_dir}",
                      file=sys.stderr)

        mod = types.ModuleType("antenv.axon_hooks")
        mod.get_axon_ntff_profile_hook = lambda: _hook
        mod.set_axon_ntff_profile_hook = lambda h: None
        sys.modules["antenv.axon_hooks"] = mod
        antenv.axon_hooks = mod
        return True
    except Exception as e:  # pragma: no cover
        print(f"ntff hook install failed: {e}", file=sys.stderr)
        return False


def bench_pjrt(in_maps, n_iters=8):
    """Run the SPMD program with device-resident inputs; return (results,
    per-iter wall times). Mirrors bass2jax.run_bass_via_pjrt but reuses the
    jitted executable and chains donated output buffers for timing."""
    import time

    import jax
    import concourse.mybir as mybir
    from jax.sharding import Mesh, PartitionSpec
    from jax.experimental.shard_map import shard_map
    from concourse import bass2jax

    nc = _get_nc()
    bass2jax.install_neuronx_cc_hook()

    part_name = nc.partition_id_tensor.name if nc.partition_id_tensor else None
    in_names, out_names, out_avals, zero_outs = [], [], [], []
    for alloc in nc.m.functions[0].allocations:
        if not isinstance(alloc, mybir.MemoryLocationSet):
            continue
        name = alloc.memorylocations[0].name
        if alloc.kind == "ExternalInput":
            if name != part_name:
                in_names.append(name)
        elif alloc.kind == "ExternalOutput":
            shape = tuple(alloc.tensor_shape)
            dtype = mybir.dt.np(alloc.dtype)
            out_names.append(name)
            out_avals.append(jax.core.ShapedArray(shape, dtype))
            zero_outs.append(np.zeros(shape, dtype))
    n_params = len(in_names)
    all_names = in_names + out_names
    if part_name is not None:
        all_names = all_names + [part_name]

    def _body(*args):
        operands = list(args)
        if part_name is not None:
            operands.append(bass2jax.partition_id_tensor())
        outs = bass2jax._bass_exec_p.bind(
            *operands,
            out_avals=tuple(out_avals),
            in_names=tuple(all_names),
            out_names=tuple(out_names),
            lowering_input_output_aliases=(),
            sim_require_finite=True,
            sim_require_nnan=True,
            nc=nc,
        )
        return tuple(outs)

    n_cores = len(in_maps)
    devices = jax.devices()[:n_cores]
    mesh = Mesh(np.asarray(devices), ("core",))
    donate = tuple(range(n_params, n_params + len(out_names)))
    sharded = jax.jit(
        shard_map(_body, mesh=mesh,
                  in_specs=(PartitionSpec("core"),) * (n_params + len(out_names)),
                  out_specs=(PartitionSpec("core"),) * len(out_names),
                  check_rep=False),
        donate_argnums=donate, keep_unused=True)

    concat_in = [
        np.concatenate([np.asarray(in_maps[c][k]) for c in range(n_cores)],
                       axis=0) for k in in_names]
    concat_zeros = [np.zeros((n_cores * z.shape[0], *z.shape[1:]), z.dtype)
                    for z in zero_outs]
    from jax.sharding import NamedSharding
    sh = NamedSharding(mesh, PartitionSpec("core"))
    dev_in = [jax.device_put(a, sh) for a in concat_in]
    outs = sharded(*dev_in, *[jax.device_put(z, sh) for z in concat_zeros])
    jax.block_until_ready(outs)
    first = [np.asarray(o) for o in outs]

    times = []
    for _ in range(n_iters):
        t0 = time.perf_counter()
        outs = sharded(*dev_in, *outs)
        jax.block_until_ready(outs)
        times.append(time.perf_counter() - t0)

    results = [
        {name: first[i].reshape(n_cores, *out_avals[i].shape)[c]
         for i, name in enumerate(out_names)}
        for c in range(n_cores)
    ]
    return results, times


def kernel(x, W_Q, W_K, W_V, W_O):
    import concourse.bass_utils as bass_utils

    x = np.asarray(x, dtype=np.float32)
    in_maps = make_in_maps(x, np.asarray(W_Q, np.float32),
                           np.asarray(W_K, np.float32),
                           np.asarray(W_V, np.float32),
                           np.asarray(W_O, np.float32))
    nc = _get_nc()
    trace = bool(int(os.environ.get("MHSA_TRACE", "0")))
    tmpdir = None
    if trace:
        trace = _ensure_ntff_hook()
    if trace:
        import tempfile
        tmpdir = tempfile.mkdtemp(prefix="mhsa_ntff_")
        _cache["trace_dir"] = tmpdir
        # no cloud creds in this container; keep artifacts local
        bass_utils.upload_artifacts = lambda d: f"local://{d}"
    res = bass_utils.run_bass_kernel_spmd(
        nc, in_maps, list(range(N_CORES)), trace=trace, tmpdir=tmpdir)
    _cache["last_results"] = res
    out = np.zeros((NT, D), dtype=np.float32)
    for r in res.results:
        out += np.asarray(r["out"], dtype=np.float32)
    return out.reshape(B, T, D)
